# revision 9
# baseline (speedup 1.0000x reference)
"""Trainium2 Bass kernel for segment-packed sliding-window linear attention
(ELU+1 feature map), one head per NeuronCore (8 heads / 8 cores).

v4 design (v3 + fixes from trace):
  * num/den accumulated TRANSPOSED: per chunk a [66, 128] PSUM slot
    (rows 0-63 = num m-dim, row 64 = den via a DEN_SC ones-column in the
    v tensor).  Every matmul streams n=128 columns:
      diag:   lhsT = vaug(i) [128t, 66],  rhs = stm(i)    [128t, 128q]
      prefix: lhsT = P(i)    [64d, 66],   rhs = qtp chunk [64d, 128q]
  * pass-1 chunk states are 32 INDEPENDENT matmuls (PE streams at full
    rate, no interleaved PSUM snapshots), batch-copied to SBUF per
    half-group; chunk prefixes P(i) + group totals built by a 3-step
    Hillis-Steele scan per 8-chunk group on DVE (adds) + GpSimd (edge
    copies), entirely off the PE queue.
  * q / k / knv inputs in fp8e4m3 (halves the DMA window); stm stays
    fp16 (PE allows mixed non-fp32 operand dtypes).
  * Causal masks generated on device (memset + affine_select).
  * Segment boundaries: extra accumulating matmuls over contiguous q
    column slices; negated slots via gpsimd scale -1; host-masked
    partial-chunk k extras (general seqlens only).  Staged seqlens cost
    3 extra 1-column matmuls.
  * Output copied out per PSUM bank (ACT/DVE alternate) and DMA'd per
    segment while later segments still compute.
"""

import numpy as np

import concourse.bass as bass
import concourse.mybir as mybir
import concourse.tile as tile
from concourse.bass_utils import run_bass_kernel_spmd

T, H, D = 4096, 8, 64
C = 128                 # chunk length
NCH = T // C            # 32 chunks
WIN = 1024
WCH = WIN // C          # 8 chunks per window
GRP = 8                 # scan group size (chunks); groups align to 1024
NG = NCH // GRP         # 4 groups
MV = 66                 # vaug slot width: 64 v + 1 den + 1 pad (4B align)
SCALE = 0.125
EPS = 1e-6
DEN_SC = 2.0 ** -6      # ones-column scale (normal in fp8e4m3, den fits fp16)
F32 = mybir.dt.float32
F16 = mybir.dt.float16
F8 = mybir.dt.float8e4
DT_Q = F8               # dtype knobs for the big input tensors
DT_K = F8               # (v must stay fp16: the output reproduces v directly
DT_KN = F8              #  at segment starts, so v's quantization error does
DT_VA = F16             #  not average out; q/k fp8 errors do)

TRACE = False           # test harness can flip for NTFF profiling
ALU = mybir.AluOpType
AF = mybir.ActivationFunctionType
WARM_MM = 3


# ----------------------------------------------------------------- host plan
def host_plan(seqlens):
    s = np.asarray(seqlens).astype(np.int64)
    assert s.shape[0] >= 2
    pos = np.arange(T)
    seg_id = np.searchsorted(s[1:], pos, side="left")
    seg_start = np.asarray(s[seg_id])
    active = seg_start < pos - WIN

    # fixed slot layout: per group g, slots 8g..8g+7 = P(8g+1..8g+7), G(g)
    slots = {}
    for g in range(NG):
        for c in range(GRP - 1):
            slots[("P", g * GRP + c + 1)] = g * GRP + c
        slots[("G", g)] = g * GRP + GRP - 1
    extra = [NCH * 1]   # next free slot index (mutable)
    extra[0] = NCH
    used = set()        # keys actually read by terms

    pps = []
    ppmap = {}
    edges = []
    edgemap = {}

    def slot(key):
        if key not in slots:
            slots[key] = extra[0]
            extra[0] += 1
        used.add(key)
        return slots[key]

    def range_terms(a, b):
        out = []
        a = max(0, min(NCH, a))
        b = max(0, min(NCH, b))
        for g in range(a // GRP, (max(b, a + 1) + GRP - 1) // GRP):
            x, y = max(a, g * GRP), min(b, (g + 1) * GRP)
            if x >= y:
                continue
            if x == g * GRP and y == (g + 1) * GRP:
                out.append((1, ("G", g)))
            elif x == g * GRP:
                out.append((1, ("P", y)))
            elif y == (g + 1) * GRP:
                out.append((1, ("G", g)))
                out.append((-1, ("P", x)))
            else:
                out.append((1, ("P", y)))
                out.append((-1, ("P", x)))
        return out

    def pp_term(sign, cb, rb):
        if rb == 0 or cb >= NCH:
            return []
        if (cb, rb) not in ppmap:
            ppmap[(cb, rb)] = len(pps)
            pps.append((cb, rb))
        return [(sign, ("PP", ppmap[(cb, rb)]))]

    chunks = []
    for i in range(NCH):
        lo = i * C
        bvals = seg_start[lo:lo + C]
        avals = active[lo:lo + C]
        variants = []
        c0 = 0
        while c0 < C:
            c1 = c0
            while (c1 < C and bvals[c1] == bvals[c0]
                   and avals[c1] == avals[c0]):
                c1 += 1
            b = int(bvals[c0])
            eidx = None
            if avals[c0]:
                raw = range_terms(i - WCH + 1, i)
                e = i - WCH
                if (i, e) not in edgemap:
                    edgemap[(i, e)] = len(edges)
                    edges.append((i, e))
                eidx = edgemap[(i, e)]
            else:
                cb, rb = b // C, b % C
                if b <= lo:
                    raw = range_terms(cb + (1 if rb else 0), i)
                    if rb:
                        raw += pp_term(-1, cb, rb)
                        raw += range_terms(cb, cb + 1)
                else:
                    raw = [(-sg, k) for (sg, k) in range_terms(i, cb)]
                    raw += pp_term(-1, cb, rb)
            terms = []
            for sg, key in raw:
                terms.append(("NEG", key) if sg < 0 else key)
            for k in terms:
                slot(k)
            variants.append(dict(c0=c0, c1=c1, terms=terms, edge=eidx))
            c0 = c1
        chunks.append(variants)

    return dict(chunks=chunks, slots=slots, used=used, pps=pps,
                edges=edges, nslot=extra[0])


def pack_head(qf, kf, v, plan):
    """qf, kf: [T, D] fp32 features; v: [T, D] fp32 -> device arrays."""
    qtp = np.ascontiguousarray(qf.T).astype(mybir.dt.np(DT_Q))     # [64, T]
    ktp = np.ascontiguousarray(kf.T).astype(mybir.dt.np(DT_K))     # [64, T]
    kn = np.ascontiguousarray(
        kf.reshape(NCH, C, D).transpose(1, 0, 2).reshape(C, NCH * D)
    ).astype(mybir.dt.np(DT_KN))
    va = np.concatenate(
        [v.reshape(NCH, C, D),
         np.full((NCH, C, 1), DEN_SC, np.float32),
         np.zeros((NCH, C, 1), np.float32)], axis=2)               # [NCH,C,66]
    va = np.ascontiguousarray(
        va.transpose(1, 0, 2).reshape(C, NCH * MV)).astype(
        mybir.dt.np(DT_VA))
    npp = max(1, len(plan["pps"]))
    kbm = np.zeros((C, npp * D), np.float32)
    for j, (cb, rb) in enumerate(plan["pps"]):
        m = (np.arange(C) < rb)[:, None]
        kbm[:, j * D:(j + 1) * D] = np.where(m, kf[cb * C:(cb + 1) * C], 0)
    return qtp, ktp, kn, va, kbm.astype(mybir.dt.np(DT_KN))


# ------------------------------------------------------------- bass program
def build_bass(plan):
    slots = plan["slots"]
    used = plan["used"]
    pps = plan["pps"]
    edges = plan["edges"]
    npp = max(1, len(pps))
    nslot = max(NCH, plan["nslot"])
    ne = max(1, len(edges))
    has_edge = len(edges) > 0

    nc = bass.Bass()
    d_q = nc.dram_tensor("qtp", [D, T], DT_Q, kind="ExternalInput")
    d_k = nc.dram_tensor("ktp", [D, T], DT_K, kind="ExternalInput")
    d_kn = nc.dram_tensor("kn", [C, NCH * D], DT_KN, kind="ExternalInput")
    d_va = nc.dram_tensor("va", [C, NCH * MV], DT_VA, kind="ExternalInput")
    d_kbm = nc.dram_tensor("kbm", [C, npp * D], DT_KN,
                           kind="ExternalInput")
    d_out = nc.dram_tensor("out", [MV, T], F16, kind="ExternalOutput")

    with tile.TileContext(nc) as tc:
        with (
            tc.tile_pool(name="persist", bufs=1) as pp_,
            tc.tile_pool(name="scratch", bufs=2) as scr_,
            tc.tile_pool(name="stp", bufs=2, space="PSUM") as stp_pool,
            tc.tile_pool(name="numb", bufs=4, space="PSUM") as num_pool,
            tc.tile_pool(name="stateb", bufs=2, space="PSUM") as st_pool,
        ):
            qtp = pp_.tile([D, T], DT_Q)
            ktp = pp_.tile([D, T], DT_K)
            kn = pp_.tile([C, NCH * D], DT_KN)
            va = pp_.tile([C, NCH * MV], DT_VA)
            kbm = pp_.tile([C, npp * D], DT_KN)
            sfl = pp_.tile([D, NCH * MV], F16)
            pf = pp_.tile([D, nslot * MV], F16)
            stm = pp_.tile([C, NCH * C], F16)
            stme = pp_.tile([C, ne * C], F16)
            ones = pp_.tile([C, 512], F16)
            trile = pp_.tile([C, 512], F16)
            trige = pp_.tile([C, 512], F16)
            osb = pp_.tile([MV, T], F16)

            def kn_ap(i):
                return kn[:, i * D:(i + 1) * D]

            def va_ap(i):
                return va[:, i * MV:(i + 1) * MV]

            def q_ap(i, c0=0, c1=C):
                return qtp[:, i * C + c0:i * C + c1]

            def k_ap(i):
                return ktp[:, i * C:(i + 1) * C]

            def pf_ap(key):
                j = slots[key]
                return pf[:, j * MV:(j + 1) * MV]

            def sfl_ap(i, n=1):
                return sfl[:, i * MV:(i + n) * MV]

            # ---------------- DMA triggers
            # ones memset early on DVE (DVE idle until first mask op)
            nc.vector.memset(ones, 1.0)
            vq = NCH * MV // NG
            th = T // 2
            # sync: kn (all), va quarters (= segments)
            nc.sync.dma_start(out=kn, in_=d_kn[:, :])
            for g in range(NG):
                nc.sync.dma_start(out=va[:, g * vq:(g + 1) * vq],
                                  in_=d_va[:, g * vq:(g + 1) * vq])
            # gpsimd: q/k halves interleaved, then mask gen
            nc.gpsimd.dma_start(out=qtp[:, :th], in_=d_q[:, :th])
            nc.gpsimd.dma_start(out=ktp[:, :th], in_=d_k[:, :th])
            nc.gpsimd.dma_start(out=qtp[:, th:], in_=d_q[:, th:])
            nc.gpsimd.dma_start(out=ktp[:, th:], in_=d_k[:, th:])
            if len(pps):
                nc.scalar.dma_start(out=kbm, in_=d_kbm[:, :])

            nc.gpsimd.affine_select(trile, ones, [[0, 4], [1, C]],
                                    ALU.is_ge, 0.0, channel_multiplier=-1)
            if has_edge:
                nc.gpsimd.affine_select(trige, ones, [[0, 4], [1, C]],
                                        ALU.is_le, 0.0, channel_multiplier=-1)

            # ---------------- PE warmup (HAM un-throttle) during DMA wait
            warm = stp_pool.tile([C, 512], F32, name="stp")
            for _ in range(WARM_MM):
                nc.tensor.matmul(warm[0:32, :], lhsT=ones[:, 0:32],
                                 rhs=ones[:, 0:512], start=True, stop=True)

            copy_engines = [nc.scalar, nc.vector]   # gpsimd cannot read PSUM

            def bank_out(bi, num_tile):
                eng = copy_engines[bi % 2]
                dst = osb[:, bi * 512:(bi + 1) * 512]
                if eng is nc.scalar:
                    eng.copy(dst, num_tile)
                else:
                    eng.tensor_copy(dst, num_tile)

            # ---------------- per-segment emission
            def pass1(g):
                # 8 independent state matmuls + 2 batch copies to SBUF
                for h in range(2):
                    i0 = g * GRP + 4 * h
                    st = st_pool.tile([D, 4 * MV], F32, name="st")
                    for x in range(4):
                        nc.tensor.matmul(st[:, x * MV:(x + 1) * MV],
                                         lhsT=kn_ap(i0 + x),
                                         rhs=va_ap(i0 + x),
                                         start=True, stop=True)
                    nc.scalar.copy(sfl_ap(i0, 4), st)
                # pp states whose source chunk lives in this group
                for j, (cb, rb) in enumerate(pps):
                    if cb // GRP == g:
                        st = st_pool.tile([D, 4 * MV], F32, name="st")
                        nc.tensor.matmul(st[:, 0:MV],
                                         lhsT=kbm[:, j * D:(j + 1) * D],
                                         rhs=va_ap(cb), start=True,
                                         stop=True)
                        key = ("PP", j)
                        if key in used:
                            nc.scalar.copy(pf_ap(key), st[:, 0:MV])
                        if ("NEG", key) in used:
                            nc.scalar.activation(pf_ap(("NEG", key)),
                                                 st[:, 0:MV], AF.Copy,
                                                 scale=-1.0)

            def scan(g):
                # inclusive Hillis-Steele over the 8 states of group g:
                # pf[8g + j] = sum_{i<=j} S(8g+i)  == P(8g+j+1) / G(g)
                s0 = g * GRP
                r1 = scr_.tile([D, GRP * MV], F16, name="r1")
                nc.gpsimd.tensor_copy(r1[:, 0:MV], sfl_ap(s0))
                nc.vector.tensor_tensor(
                    r1[:, MV:GRP * MV], sfl_ap(s0 + 1, GRP - 1),
                    sfl_ap(s0, GRP - 1), ALU.add)
                dst = pf[:, s0 * MV:(s0 + GRP) * MV]
                nc.gpsimd.tensor_copy(dst[:, 0:2 * MV], r1[:, 0:2 * MV])
                nc.vector.tensor_tensor(
                    dst[:, 2 * MV:GRP * MV], r1[:, 2 * MV:GRP * MV],
                    r1[:, 0:(GRP - 2) * MV], ALU.add)
                # final in-place step: disjoint src/dst halves
                nc.vector.tensor_tensor(
                    dst[:, 4 * MV:GRP * MV], dst[:, 4 * MV:GRP * MV],
                    dst[:, 0:4 * MV], ALU.add)
                # negated twins needed from this group
                for c in range(GRP):
                    key = (("P", s0 + c + 1) if c < GRP - 1 else ("G", g))
                    nkey = ("NEG", key)
                    if nkey in used:
                        nc.gpsimd.tensor_scalar_mul(
                            pf_ap(nkey), pf_ap(key), -1.0)

            def scores(g):
                for h in range(2):
                    i0 = g * GRP + 4 * h
                    stp = stp_pool.tile([C, 512], F32, name="stp")
                    for x in range(4):
                        nc.tensor.matmul(stp[:, x * C:(x + 1) * C],
                                         lhsT=k_ap(i0 + x),
                                         rhs=q_ap(i0 + x),
                                         start=True, stop=True)
                    nc.vector.scalar_tensor_tensor(
                        stm[:, i0 * C:(i0 + 4) * C], stp, 1.0, trile,
                        ALU.bypass, ALU.mult)
                todo = [(xi, ei) for (xi, (qi, ei)) in enumerate(edges)
                        if qi // GRP == g]
                for x0 in range(0, len(todo), 4):
                    batch = todo[x0:x0 + 4]
                    stp = stp_pool.tile([C, 512], F32, name="stp")
                    for x, (xi, ei) in enumerate(batch):
                        qi = edges[xi][0]
                        nc.tensor.matmul(stp[:, x * C:(x + 1) * C],
                                         lhsT=k_ap(ei), rhs=q_ap(qi),
                                         start=True, stop=True)
                    w = len(batch) * C
                    nc.vector.scalar_tensor_tensor(
                        stme[:, batch[0][0] * C:batch[0][0] * C + w],
                        stp[:, :w], 1.0, trige[:, :w],
                        ALU.bypass, ALU.mult)

            def numwork(g):
                for h in range(2):
                    i0 = g * GRP + 4 * h
                    bi = 2 * g + h
                    numt = num_pool.tile([MV, 512], F32, name="numt")
                    for x in range(4):
                        i = i0 + x
                        slot = numt[:, x * C:(x + 1) * C]
                        mms = [("diag", None, 0, C)]
                        for v in plan["chunks"][i]:
                            for key in v["terms"]:
                                mms.append(("term", key, v["c0"], v["c1"]))
                            if v["edge"] is not None:
                                mms.append(("edge", v["edge"],
                                            v["c0"], v["c1"]))
                        for mi, (kind, key, c0, c1) in enumerate(mms):
                            last = mi == len(mms) - 1
                            if kind == "diag":
                                nc.tensor.matmul(
                                    slot, lhsT=va_ap(i),
                                    rhs=stm[:, i * C:(i + 1) * C],
                                    start=True, stop=last,
                                    skip_group_check=True)
                            elif kind == "term":
                                nc.tensor.matmul(
                                    slot[:, c0:c1], lhsT=pf_ap(key),
                                    rhs=q_ap(i, c0, c1),
                                    start=False, stop=last,
                                    skip_group_check=True)
                            else:
                                ei = edges[key][1]
                                nc.tensor.matmul(
                                    slot[:, c0:c1], lhsT=va_ap(ei),
                                    rhs=stme[:, key * C + c0:key * C + c1],
                                    start=False, stop=last,
                                    skip_group_check=True)
                    bank_out(bi, numt)
                dq = [nc.sync, nc.gpsimd, nc.sync, nc.gpsimd][g]
                dq.dma_start(out=d_out[:, g * 1024:(g + 1) * 1024],
                             in_=osb[:, g * 1024:(g + 1) * 1024])

            # numwork(s) may read slots written by pass1/scan of a later
            # group (degenerate segments reference forward chunks).
            def dep_group(s):
                dep = s
                for i in range(s * GRP, (s + 1) * GRP):
                    for v in plan["chunks"][i]:
                        for key in v["terms"]:
                            k = key[1] if key[0] == "NEG" else key
                            if k[0] == "P":
                                dep = max(dep, (k[1] - 1) // GRP)
                            elif k[0] == "G":
                                dep = max(dep, k[1])
                            else:
                                dep = max(dep, pps[k[1]][0] // GRP)
                return dep

            pending = list(range(NG))
            for g in range(NG):
                pass1(g)
                scan(g)
                scores(g)
                if g < NG - 1:
                    for s in [s for s in pending]:
                        if s < g and dep_group(s) <= g:
                            numwork(s)
                            pending.remove(s)
            for s in pending:
                numwork(s)
    return nc


def split_waits(bir: bytes) -> bytes:
    """Walrus codegen caps sync waits at 1 per instruction (2 for
    EventSemaphore); Tile sometimes attaches more.  Hoist the excess into
    preceding same-engine NoOps (engines are in-order, so semantics hold)."""
    import json
    m = json.loads(bir)
    for f in m["functions"]:
        for bb in f["blocks"]:
            out = []
            for ins in bb["instructions"]:
                si = ins.get("sync_info")
                ow = (si or {}).get("on_wait") or []
                cap = 2 if ins.get("opcode") == "EventSemaphore" else 1
                eng = ins.get("engine")
                if eng and len(ow) > cap:
                    keep = ow[-cap:]
                    for j, w in enumerate(ow[:-cap]):
                        out.append({"name": f'{ins["name"]}_sw{j}',
                                    "opcode": "NoOp", "engine": eng,
                                    "ins": [], "outs": [],
                                    "sync_info": {"on_wait": [w],
                                                  "on_update": []}})
                    ins = dict(ins)
                    ins["sync_info"] = {
                        "on_wait": keep,
                        "on_update": (si or {}).get("on_update") or []}
                out.append(ins)
            bb["instructions"] = out
    return json.dumps(m).encode()


# ------------------------------------------------------------------ driver
def elu(x):
    return np.where(x > 0, x, np.expm1(np.minimum(x, 0.0)))


def kernel(**inputs):
    q = np.asarray(inputs["q"], dtype=np.float32)
    k = np.asarray(inputs["k"], dtype=np.float32)
    v = np.asarray(inputs["v"], dtype=np.float32)
    seqlens = np.asarray(inputs["seqlens"])
    assert q.shape == (T, H, D), q.shape

    qf = elu(q * SCALE) + 1.0
    kf = elu(k) + 1.0

    plan = host_plan(seqlens)
    nc = build_bass(plan)
    patched = split_waits(nc.to_json_bytes())
    nc.to_json_bytes = lambda: patched

    in_maps = []
    for h in range(H):
        qtp, ktp, kn, va, kbm = pack_head(qf[:, h], kf[:, h], v[:, h], plan)
        in_maps.append(dict(qtp=qtp, ktp=ktp, kn=kn, va=va, kbm=kbm))

    res = run_bass_kernel_spmd(nc, in_maps, core_ids=list(range(H)),
                               trace=TRACE)
    if TRACE:
        kernel.last_result = res
    out = np.empty((T, H, D), np.float32)
    for h in range(H):
        raw = np.asarray(res.results[h]["out"], dtype=np.float32)  # [66, T]
        den = np.maximum(raw[D] / DEN_SC, EPS)
        out[:, h, :] = (raw[:D] / den).T
    return out


# revision 12
# speedup vs baseline: 1.3097x; 1.3097x over previous
"""Trainium2 Bass kernel for segment-packed sliding-window linear attention
(ELU+1 feature map), one head per NeuronCore (8 heads / 8 cores).

v5 design (what the traces taught us):
  * Matmuls with 128-column fp8 weights hit the fast-weight-load path and
    stream back-to-back at ~1 ns/col; 66-col fp16 weights stall ~165 ns
    per matmul.  So every stationary operand is a 128-col fp8 tensor:
      pass1:  lhsT = kn(i)  [128t, 64] f8, rhs = va(i) [128t, 66] f16
      score:  lhsT = ktp(i) [64d, 128] f8, rhs = qtp(i) [64d, 128] f8
      diag:   lhsT = stm(i) [128t,128] f8, rhs = va(i)  [128t, 66] f16
      prefix: lhsT = qtp(i) [64d, 128] f8, rhs = P(i)   [64d, 66] f16
    num/den accumulate per chunk into [128q, 66] PSUM slots (col 64 = den
    via a DEN_SC ones-column in va).
  * v (and the va tensor) stays fp16: the output reproduces v directly at
    segment starts, so v's quantization error does not average out.  q, k
    and the masked scores stm are fp8 (their errors average in the
    contractions; verified in CoreSim).
  * pass-1 chunk states are independent matmuls batch-copied to SBUF;
    prefixes P(i) + group totals via a 3-step Hillis-Steele scan per
    8-chunk group on DVE/GpSimd, off the PE queue.
  * Causal masks generated on device (memset + affine_select); masked
    score copies PSUM->SBUF are fused with the mask multiply on DVE.
  * Segment-boundary variants use host-masked q copies as lhsT (the out
    partition range of a matmul must start at a 32-multiple, so variant
    q-row ranges are expressed by zeroing lhsT columns on the host).
  * Output copied out per PSUM bank and DMA'd per segment while later
    segments still compute.
"""

import numpy as np

import concourse.bass as bass
import concourse.mybir as mybir
import concourse.tile as tile
from concourse.bass_utils import run_bass_kernel_spmd

T, H, D = 4096, 8, 64
C = 128                 # chunk length
NCH = T // C            # 32 chunks
WIN = 1024
WCH = WIN // C          # 8 chunks per window
GRP = 8                 # scan group size (chunks); groups align to 1024
NG = NCH // GRP         # 4 groups
MV = 66                 # va / num slot width: 64 v + 1 den + 1 pad
SCALE = 0.125
EPS = 1e-6
DEN_SC = 2.0 ** -6      # ones-column scale (normal in fp8e4m3, den fits f16)
F32 = mybir.dt.float32
F16 = mybir.dt.float16
F8 = mybir.dt.float8e4
DT_Q = F8
DT_K = F8
DT_KN = F8
DT_VA = F16
DT_STM = F16

TRACE = False           # test harness can flip for NTFF profiling
ALU = mybir.AluOpType
AF = mybir.ActivationFunctionType
WARM_MM = 3


# ----------------------------------------------------------------- host plan
def host_plan(seqlens):
    s = np.asarray(seqlens).astype(np.int64)
    assert s.shape[0] >= 2
    pos = np.arange(T)
    seg_id = np.searchsorted(s[1:], pos, side="left")
    seg_start = np.asarray(s[seg_id])
    active = seg_start < pos - WIN

    # fixed slot layout: per group g, slots 8g..8g+7 = P(8g+1..8g+7), G(g)
    slots = {}
    for g in range(NG):
        for c in range(GRP - 1):
            slots[("P", g * GRP + c + 1)] = g * GRP + c
        slots[("G", g)] = g * GRP + GRP - 1
    extra = [NCH]
    used = set()

    pps = []
    ppmap = {}
    edges = []
    edgemap = {}
    qvars = []          # (chunk, c0, c1) host-masked q lhsT tiles

    def slot(key):
        if key not in slots:
            slots[key] = extra[0]
            extra[0] += 1
        used.add(key)
        return slots[key]

    def range_terms(a, b):
        out = []
        a = max(0, min(NCH, a))
        b = max(0, min(NCH, b))
        for g in range(a // GRP, (max(b, a + 1) + GRP - 1) // GRP):
            x, y = max(a, g * GRP), min(b, (g + 1) * GRP)
            if x >= y:
                continue
            if x == g * GRP and y == (g + 1) * GRP:
                out.append((1, ("G", g)))
            elif x == g * GRP:
                out.append((1, ("P", y)))
            elif y == (g + 1) * GRP:
                out.append((1, ("G", g)))
                out.append((-1, ("P", x)))
            else:
                out.append((1, ("P", y)))
                out.append((-1, ("P", x)))
        return out

    def pp_term(sign, cb, rb):
        if rb == 0 or cb >= NCH:
            return []
        if (cb, rb) not in ppmap:
            ppmap[(cb, rb)] = len(pps)
            pps.append((cb, rb))
        return [(sign, ("PP", ppmap[(cb, rb)]))]

    chunks = []
    for i in range(NCH):
        lo = i * C
        bvals = seg_start[lo:lo + C]
        avals = active[lo:lo + C]
        variants = []
        c0 = 0
        while c0 < C:
            c1 = c0
            while (c1 < C and bvals[c1] == bvals[c0]
                   and avals[c1] == avals[c0]):
                c1 += 1
            b = int(bvals[c0])
            eidx = None
            if avals[c0]:
                raw = range_terms(i - WCH + 1, i)
                e = i - WCH
                if (i, e) not in edgemap:
                    edgemap[(i, e)] = len(edges)
                    edges.append((i, e))
                eidx = edgemap[(i, e)]
            else:
                cb, rb = b // C, b % C
                if b <= lo:
                    raw = range_terms(cb + (1 if rb else 0), i)
                    if rb:
                        raw += pp_term(-1, cb, rb)
                        raw += range_terms(cb, cb + 1)
                else:
                    raw = [(-sg, k) for (sg, k) in range_terms(i, cb)]
                    raw += pp_term(-1, cb, rb)
            terms = []
            for sg, key in raw:
                terms.append(("NEG", key) if sg < 0 else key)
            for k in terms:
                slot(k)
            qv = None
            if terms and not (c0 == 0 and c1 == C):
                qv = len(qvars)
                qvars.append((i, c0, c1))
            variants.append(dict(c0=c0, c1=c1, terms=terms, edge=eidx,
                                 qv=qv))
            c0 = c1
        chunks.append(variants)

    return dict(chunks=chunks, slots=slots, used=used, pps=pps,
                edges=edges, qvars=qvars, nslot=extra[0])


def pack_head(qf, kf, v, plan):
    """qf, kf: [T, D] fp32 features; v: [T, D] fp32 -> device arrays."""
    qtp = np.ascontiguousarray(qf.T).astype(mybir.dt.np(DT_Q))     # [64, T]
    ktp = np.ascontiguousarray(kf.T).astype(mybir.dt.np(DT_K))     # [64, T]
    kn = np.ascontiguousarray(
        kf.reshape(NCH, C, D).transpose(1, 0, 2).reshape(C, NCH * D)
    ).astype(mybir.dt.np(DT_KN))
    va = np.concatenate(
        [v.reshape(NCH, C, D),
         np.full((NCH, C, 1), DEN_SC, np.float32),
         np.zeros((NCH, C, 1), np.float32)], axis=2)               # [NCH,C,66]
    va = np.ascontiguousarray(
        va.transpose(1, 0, 2).reshape(C, NCH * MV)).astype(
        mybir.dt.np(DT_VA))
    npp = max(1, len(plan["pps"]))
    kbm = np.zeros((C, npp * D), np.float32)
    for j, (cb, rb) in enumerate(plan["pps"]):
        m = (np.arange(C) < rb)[:, None]
        kbm[:, j * D:(j + 1) * D] = np.where(m, kf[cb * C:(cb + 1) * C], 0)
    nqv = max(1, len(plan["qvars"]))
    qvm = np.zeros((D, nqv * C), np.float32)
    for j, (i, c0, c1) in enumerate(plan["qvars"]):
        qvm[:, j * C + c0:j * C + c1] = qf.T[:, i * C + c0:i * C + c1]
    return (qtp, ktp, kn, va, kbm.astype(mybir.dt.np(DT_KN)),
            qvm.astype(mybir.dt.np(DT_Q)))


# ------------------------------------------------------------- bass program
def build_bass(plan):
    slots = plan["slots"]
    used = plan["used"]
    pps = plan["pps"]
    edges = plan["edges"]
    qvars = plan["qvars"]
    npp = max(1, len(pps))
    nqv = max(1, len(qvars))
    nslot = max(NCH, plan["nslot"])
    ne = max(1, len(edges))
    has_edge = len(edges) > 0

    nc = bass.Bass()
    d_q = nc.dram_tensor("qtp", [D, T], DT_Q, kind="ExternalInput")
    d_k = nc.dram_tensor("ktp", [D, T], DT_K, kind="ExternalInput")
    d_kn = nc.dram_tensor("kn", [C, NCH * D], DT_KN, kind="ExternalInput")
    d_va = nc.dram_tensor("va", [C, NCH * MV], DT_VA, kind="ExternalInput")
    d_kbm = nc.dram_tensor("kbm", [C, npp * D], DT_KN,
                           kind="ExternalInput")
    d_qvm = nc.dram_tensor("qvm", [D, nqv * C], DT_Q,
                           kind="ExternalInput")
    d_out = nc.dram_tensor("out", [C, NCH * MV], F16, kind="ExternalOutput")

    with tile.TileContext(nc) as tc:
        with (
            tc.tile_pool(name="persist", bufs=1) as pp_,
            tc.tile_pool(name="scratch", bufs=2) as scr_,
            tc.tile_pool(name="stp", bufs=2, space="PSUM") as stp_pool,
            tc.tile_pool(name="numb", bufs=4, space="PSUM") as num_pool,
            tc.tile_pool(name="stateb", bufs=2, space="PSUM") as st_pool,
        ):
            qtp = pp_.tile([D, T], DT_Q)
            ktp = pp_.tile([D, T], DT_K)
            kn = pp_.tile([C, NCH * D], DT_KN)
            va = pp_.tile([C, NCH * MV], DT_VA)
            kbm = pp_.tile([C, npp * D], DT_KN)
            qvm = pp_.tile([D, nqv * C], DT_Q)
            sfl = pp_.tile([D, NCH * MV], F16)
            pf = pp_.tile([D, nslot * MV], F16)
            stm = pp_.tile([C, NCH * C], DT_STM)
            stme = pp_.tile([C, ne * C], DT_STM)
            ones = pp_.tile([C, 512], F16)
            trile = pp_.tile([C, 512], F16)
            trige = pp_.tile([C, 512], F16)
            osb = pp_.tile([C, NCH * MV], F16)

            def kn_ap(i):
                return kn[:, i * D:(i + 1) * D]

            def va_ap(i):
                return va[:, i * MV:(i + 1) * MV]

            def q_ap(i):
                return qtp[:, i * C:(i + 1) * C]

            def k_ap(i):
                return ktp[:, i * C:(i + 1) * C]

            def pf_ap(key):
                j = slots[key]
                return pf[:, j * MV:(j + 1) * MV]

            def sfl_ap(i, n=1):
                return sfl[:, i * MV:(i + n) * MV]

            # ---------------- DMA triggers
            nc.vector.memset(ones, 1.0)
            # dummy ACT op: forces the one-time ACT_TABLE_LOAD (~1.5us)
            # during the DMA wait instead of before the first state copy
            nc.scalar.copy(trile[0:1, 0:1], ones[0:1, 0:1])
            vq = NCH * MV // NG
            th = T // 2
            # sync: kn (all), va quarters (= segments)
            nc.sync.dma_start(out=kn, in_=d_kn[:, :])
            for g in range(NG):
                nc.sync.dma_start(out=va[:, g * vq:(g + 1) * vq],
                                  in_=d_va[:, g * vq:(g + 1) * vq])
            # gpsimd: q/k halves interleaved, then mask gen
            nc.gpsimd.dma_start(out=qtp[:, :th], in_=d_q[:, :th])
            nc.gpsimd.dma_start(out=ktp[:, :th], in_=d_k[:, :th])
            nc.gpsimd.dma_start(out=qtp[:, th:], in_=d_q[:, th:])
            nc.gpsimd.dma_start(out=ktp[:, th:], in_=d_k[:, th:])
            if len(pps):
                nc.scalar.dma_start(out=kbm, in_=d_kbm[:, :])
            if len(qvars):
                nc.scalar.dma_start(out=qvm, in_=d_qvm[:, :])

            nc.gpsimd.affine_select(trile, ones, [[0, 4], [1, C]],
                                    ALU.is_ge, 0.0, channel_multiplier=-1)
            if has_edge:
                nc.gpsimd.affine_select(trige, ones, [[0, 4], [1, C]],
                                        ALU.is_le, 0.0, channel_multiplier=-1)
                nc.gpsimd.memset(stme, 0.0)

            # ---------------- PE warmup (HAM un-throttle) during DMA wait
            warm = stp_pool.tile([C, 512], F32, name="stp")
            for _ in range(WARM_MM):
                nc.tensor.matmul(warm[0:32, :], lhsT=ones[:, 0:32],
                                 rhs=ones[:, 0:512], start=True, stop=True)

            copy_engines = [nc.scalar, nc.vector]   # gpsimd cannot read PSUM

            def psum_copy(eng, dst, src):
                if eng is nc.scalar:
                    eng.copy(dst, src)
                else:
                    eng.tensor_copy(dst, src)

            # ---------------- per-segment emission
            def pass1(g):
                for h in range(2):
                    i0 = g * GRP + 4 * h
                    st = st_pool.tile([D, 4 * MV], F32, name="st")
                    for x in range(4):
                        nc.tensor.matmul(st[:, x * MV:(x + 1) * MV],
                                         lhsT=kn_ap(i0 + x),
                                         rhs=va_ap(i0 + x),
                                         start=True, stop=True)
                    nc.scalar.copy(sfl_ap(i0, 4), st)
                for j, (cb, rb) in enumerate(pps):
                    if cb // GRP == g:
                        st = st_pool.tile([D, 4 * MV], F32, name="st")
                        nc.tensor.matmul(st[:, 0:MV],
                                         lhsT=kbm[:, j * D:(j + 1) * D],
                                         rhs=va_ap(cb), start=True,
                                         stop=True)
                        key = ("PP", j)
                        if key in used:
                            nc.scalar.copy(pf_ap(key), st[:, 0:MV])
                        if ("NEG", key) in used:
                            nc.scalar.activation(pf_ap(("NEG", key)),
                                                 st[:, 0:MV], AF.Copy,
                                                 scale=-1.0)

            def scan(g):
                # pf[8g + j] = sum_{i<=j} S(8g+i)  == P(8g+j+1) / G(g)
                s0 = g * GRP
                r1 = scr_.tile([D, GRP * MV], F16, name="r1")
                nc.gpsimd.tensor_copy(r1[:, 0:MV], sfl_ap(s0))
                nc.vector.tensor_tensor(
                    r1[:, MV:GRP * MV], sfl_ap(s0 + 1, GRP - 1),
                    sfl_ap(s0, GRP - 1), ALU.add)
                dst = pf[:, s0 * MV:(s0 + GRP) * MV]
                nc.gpsimd.tensor_copy(dst[:, 0:2 * MV], r1[:, 0:2 * MV])
                nc.vector.tensor_tensor(
                    dst[:, 2 * MV:GRP * MV], r1[:, 2 * MV:GRP * MV],
                    r1[:, 0:(GRP - 2) * MV], ALU.add)
                nc.vector.tensor_tensor(
                    dst[:, 4 * MV:GRP * MV], dst[:, 4 * MV:GRP * MV],
                    dst[:, 0:4 * MV], ALU.add)
                for c in range(GRP):
                    key = (("P", s0 + c + 1) if c < GRP - 1 else ("G", g))
                    nkey = ("NEG", key)
                    if nkey in used:
                        nc.gpsimd.tensor_scalar_mul(
                            pf_ap(nkey), pf_ap(key), -1.0)

            def scores(g):
                for h in range(2):
                    i0 = g * GRP + 4 * h
                    stp = stp_pool.tile([C, 512], F32, name="stp")
                    for x in range(4):
                        nc.tensor.matmul(stp[:, x * C:(x + 1) * C],
                                         lhsT=k_ap(i0 + x),
                                         rhs=q_ap(i0 + x),
                                         start=True, stop=True)
                    nc.vector.scalar_tensor_tensor(
                        stm[:, i0 * C:(i0 + 4) * C], stp, 1.0, trile,
                        ALU.bypass, ALU.mult)
                todo = [(xi, ei) for (xi, (qi, ei)) in enumerate(edges)
                        if qi // GRP == g]
                for x0 in range(0, len(todo), 4):
                    batch = todo[x0:x0 + 4]
                    stp = stp_pool.tile([C, 512], F32, name="stp")
                    for x, (xi, ei) in enumerate(batch):
                        nc.tensor.matmul(stp[:, x * C:(x + 1) * C],
                                         lhsT=k_ap(ei),
                                         rhs=q_ap(edges[xi][0]),
                                         start=True, stop=True)
                    # per-edge masked copy, column-sliced to the active
                    # variant q-range (stme was memset to 0)
                    for x, (xi, ei) in enumerate(batch):
                        qi = edges[xi][0]
                        for v in plan["chunks"][qi]:
                            if v["edge"] != xi:
                                continue
                            c0, c1 = v["c0"], v["c1"]
                            nc.vector.scalar_tensor_tensor(
                                stme[:, xi * C + c0:xi * C + c1],
                                stp[:, x * C + c0:x * C + c1], 1.0,
                                trige[:, x * C + c0:x * C + c1],
                                ALU.bypass, ALU.mult)

            def numwork(g):
                for h in range(2):
                    i0 = g * GRP + 4 * h
                    numt = num_pool.tile([C, 4 * MV], F32, name="numt")
                    for x in range(4):
                        i = i0 + x
                        slot = numt[:, x * MV:(x + 1) * MV]
                        mms = [("diag", None, None)]
                        for v in plan["chunks"][i]:
                            lh = None if v["qv"] is None else v["qv"]
                            for key in v["terms"]:
                                mms.append(("term", key, lh))
                            if v["edge"] is not None:
                                mms.append(("edge", v["edge"], None))
                        for mi, (kind, key, lh) in enumerate(mms):
                            last = mi == len(mms) - 1
                            if kind == "diag":
                                nc.tensor.matmul(
                                    slot, lhsT=stm[:, i * C:(i + 1) * C],
                                    rhs=va_ap(i), start=True, stop=last,
                                    skip_group_check=True)
                            elif kind == "term":
                                lhsT = (q_ap(i) if lh is None
                                        else qvm[:, lh * C:(lh + 1) * C])
                                nc.tensor.matmul(
                                    slot, lhsT=lhsT, rhs=pf_ap(key),
                                    start=False, stop=last,
                                    skip_group_check=True)
                            else:
                                nc.tensor.matmul(
                                    slot,
                                    lhsT=stme[:, key * C:(key + 1) * C],
                                    rhs=va_ap(edges[key][1]),
                                    start=False, stop=last,
                                    skip_group_check=True)
                    psum_copy(copy_engines[(2 * g + h) % 2],
                              osb[:, i0 * MV:(i0 + 4) * MV], numt)
                dq = [nc.sync, nc.gpsimd, nc.sync, nc.gpsimd][g]
                dq.dma_start(
                    out=d_out[:, g * GRP * MV:(g + 1) * GRP * MV],
                    in_=osb[:, g * GRP * MV:(g + 1) * GRP * MV])

            def dep_group(s):
                dep = s
                for i in range(s * GRP, (s + 1) * GRP):
                    for v in plan["chunks"][i]:
                        for key in v["terms"]:
                            k = key[1] if key[0] == "NEG" else key
                            if k[0] == "P":
                                dep = max(dep, (k[1] - 1) // GRP)
                            elif k[0] == "G":
                                dep = max(dep, k[1])
                            else:
                                dep = max(dep, pps[k[1]][0] // GRP)
                return dep

            pending = list(range(NG))
            for g in range(NG):
                pass1(g)
                scan(g)
                scores(g)
                if g < NG - 1:
                    for s in [s for s in pending]:
                        if s < g and dep_group(s) <= g:
                            numwork(s)
                            pending.remove(s)
            for s in pending:
                numwork(s)
    return nc


def split_waits(bir: bytes) -> bytes:
    """Walrus codegen caps sync waits at 1 per instruction (2 for
    EventSemaphore); Tile sometimes attaches more.  Hoist the excess into
    preceding same-engine NoOps (engines are in-order, so semantics hold)."""
    import json
    m = json.loads(bir)
    for f in m["functions"]:
        for bb in f["blocks"]:
            out = []
            for ins in bb["instructions"]:
                si = ins.get("sync_info")
                ow = (si or {}).get("on_wait") or []
                cap = 2 if ins.get("opcode") == "EventSemaphore" else 1
                eng = ins.get("engine")
                if eng and len(ow) > cap:
                    keep = ow[-cap:]
                    for j, w in enumerate(ow[:-cap]):
                        out.append({"name": f'{ins["name"]}_sw{j}',
                                    "opcode": "NoOp", "engine": eng,
                                    "ins": [], "outs": [],
                                    "sync_info": {"on_wait": [w],
                                                  "on_update": []}})
                    ins = dict(ins)
                    ins["sync_info"] = {
                        "on_wait": keep,
                        "on_update": (si or {}).get("on_update") or []}
                out.append(ins)
            bb["instructions"] = out
    return json.dumps(m).encode()


# ------------------------------------------------------------------ driver
def elu(x):
    return np.where(x > 0, x, np.expm1(np.minimum(x, 0.0)))


def kernel(**inputs):
    q = np.asarray(inputs["q"], dtype=np.float32)
    k = np.asarray(inputs["k"], dtype=np.float32)
    v = np.asarray(inputs["v"], dtype=np.float32)
    seqlens = np.asarray(inputs["seqlens"])
    assert q.shape == (T, H, D), q.shape

    qf = elu(q * SCALE) + 1.0
    kf = elu(k) + 1.0

    plan = host_plan(seqlens)
    nc = build_bass(plan)
    patched = split_waits(nc.to_json_bytes())
    nc.to_json_bytes = lambda: patched

    in_maps = []
    for h in range(H):
        qtp, ktp, kn, va, kbm, qvm = pack_head(
            qf[:, h], kf[:, h], v[:, h], plan)
        in_maps.append(dict(qtp=qtp, ktp=ktp, kn=kn, va=va, kbm=kbm,
                            qvm=qvm))

    res = run_bass_kernel_spmd(nc, in_maps, core_ids=list(range(H)),
                               trace=TRACE)
    if TRACE:
        kernel.last_result = res
    out = np.empty((T, H, D), np.float32)
    for h in range(H):
        raw = np.asarray(res.results[h]["out"],
                         dtype=np.float32)          # [128, NCH*66]
        for cchunk in range(NCH):
            sl = raw[:, cchunk * MV:cchunk * MV + D + 1]
            den = np.maximum(sl[:, D] / DEN_SC, EPS)
            out[cchunk * C:(cchunk + 1) * C, h, :] = sl[:, :D] / den[:, None]
    return out
